# revision 7
# baseline (speedup 1.0000x reference)
"""Trainium2 Bass kernel for nn_Attn_44281112822483.

Computes attn = softmax_s( hidden[0,b,:] . (W @ encoder_outputs[s,b,:] + bias) )
for S=4096, B=64, H=300, returning [1, B, S] float32.

Key algebraic reduction: with u = hidden[0] @ W  ([B, H]),
    energies[b, s] = sum_h encoder_outputs[s, b, h] * u[b, h] + const(b)
and the per-row constant (hidden . bias) cancels inside the softmax, so the
device only needs one streaming pass over encoder_outputs (315 MB) — a
memory-bound batched dot — plus a tiny [B,H]x[H,H] matmul and a softmax.

Sharding: data-parallel over batch across 8 NeuronCores (8 batches/core);
the 300x300 weight is replicated. Each core's encoder slice is packed
host-side as [8 batches x 320 padded h-rows, 4096 s] so every HBM->SBUF DMA
is a full 128-partition tile with 4 KB contiguous rows and every matmul
K-segment is 32-aligned.
"""

import os
import numpy as np

S = 4096
B = 64
H = 300
NCORES = 8
BL = B // NCORES          # batches per core = 8
HP = 320                  # per-batch padded row count (multiple of 64)
R = BL * HP               # 2560 packed rows per core
GT = R // 128             # 20 row-tiles of 128
W_DMA = 1024              # dma tile width (columns of s)
NS = 512                  # matmul moving free dim (one PSUM bank of fp32)

# fp32r: single-pass reduced-precision fp32 matmul (1 cycle/row at N>=256).
# Plain fp32 is exact but 4 cycles/row. Toggle if accuracy demands.
USE_F32R = os.environ.get("BASS_KERNEL_F32", "") == ""

_cache = {}


def _segments(g):
    """Segments of row-tile g: list of (batch, partition offset, length)."""
    lo, hi = g * 128, (g + 1) * 128
    segs = []
    b0 = lo // HP
    for b in (b0, b0 + 1):
        if b >= BL:
            break
        s, e = max(lo, b * HP), min(hi, (b + 1) * HP)
        if s < e:
            segs.append((b, s - lo, e - s))
    return segs


def _g_first(b):
    return (b * HP) // 128


def _g_last(b):
    return (b * HP + HP - 1) // 128


def _build_nc():
    import concourse.bass as bass
    import concourse.tile as tile
    from concourse import bacc, mybir
    from contextlib import ExitStack

    f32 = mybir.dt.float32
    fmm = mybir.dt.float32r if USE_F32R else mybir.dt.float32

    nc = bacc.Bacc(
        "TRN2",
        target_bir_lowering=False,
        debug=False,
        enable_asserts=True,
        num_devices=NCORES,
    )
    eop = nc.dram_tensor("eop", [R, S], fmm, kind="ExternalInput").ap()
    ht = nc.dram_tensor("ht", [H, BL], f32, kind="ExternalInput").ap()
    wm = nc.dram_tensor("wm", [H, H], f32, kind="ExternalInput").ap()
    attn = nc.dram_tensor("attn", [BL, S], f32, kind="ExternalOutput").ap()

    KC = [(0, 128), (128, 128), (256, H - 256)]  # contraction chunks over o

    with tile.TileContext(nc) as tc:
        with ExitStack() as ctx:
            consts = ctx.enter_context(tc.tile_pool(name="consts", bufs=1))
            upool = ctx.enter_context(tc.tile_pool(name="upool", bufs=1))
            upsum = ctx.enter_context(
                tc.tile_pool(name="upsum", bufs=1, space="PSUM")
            )
            epool = ctx.enter_context(tc.tile_pool(name="epool", bufs=8))
            psum = ctx.enter_context(tc.tile_pool(name="psum", bufs=6, space="PSUM"))
            persist = ctx.enter_context(tc.tile_pool(name="persist", bufs=1))
            stats = ctx.enter_context(tc.tile_pool(name="stats", bufs=1))

            # ---- load consts -------------------------------------------------
            ht_t, wm_t = [], []
            for ci, (o0, kc) in enumerate(KC):
                t = consts.tile([128, BL], f32, tag=f"ht{ci}")
                nc.sync.dma_start(t[:kc, :], ht[o0:o0 + kc, :])
                ht_t.append(t)
                t = consts.tile([128, H], f32, tag=f"wm{ci}")
                nc.sync.dma_start(t[:kc, :], wm[o0:o0 + kc, :])
                wm_t.append(t)

            # Pre-warm the ACT exp table (the PSEUDO_LOAD_ACT_FUNC_SET DMA
            # runs once, early, hidden under the encoder stream).
            warm = stats.tile([1, 1], f32, tag="warm")
            nc.vector.memset(warm[:, :], 0.0)
            nc.scalar.activation(
                warm[:, :], warm[:, :], mybir.ActivationFunctionType.Exp
            )

            # ---- u = hidden @ W  -> u_sb [BL, H] (exact fp32) ---------------
            u_ps = upsum.tile([BL, H], f32, tag="ups")
            for ci, (o0, kc) in enumerate(KC):
                nc.tensor.matmul(
                    u_ps[:, :],
                    ht_t[ci][:kc, :],
                    wm_t[ci][:kc, :],
                    start=(ci == 0),
                    stop=(ci == len(KC) - 1),
                )
            u_sb = upool.tile([BL, H], f32, tag="usb")
            nc.vector.tensor_copy(u_sb[:, :], u_ps[:, :])

            # ---- scatter u into packed layout [128, GT, BL] -----------------
            # u_pack[p, g, j] = u[b, h] when row 128g+p is (b, h) and j == b,
            # else 0.  Column-selective zeros let one PSUM tile accumulate all
            # batches: row j of (lhsT.T @ rhs) sums only batch j's rows.
            u_pack = upool.tile([128, GT, BL], fmm, tag="upack")
            nc.vector.memset(u_pack[:, :, :].bitcast(f32), 0.0)
            for b in range(BL):
                h0 = 0
                while h0 < H:
                    r0 = b * HP + h0
                    g, off = divmod(r0, 128)
                    ln = min(128 - off, H - h0)
                    src = u_sb[b:b + 1, h0:h0 + ln].bitcast(fmm)
                    nc.sync.dma_start(u_pack[off:off + ln, g, b:b + 1], src)
                    h0 += ln

            # ---- main stream: energies[b, s] ---------------------------------
            # One full-K=128 matmul per (row-tile, 512-col slice): the
            # column-selective zeros in u_pack route each packed row's
            # contribution to its own batch's PSUM row, so batch boundaries
            # inside a row-tile need no splitting.
            energies = persist.tile([BL, S], f32, tag="energies")
            for nb in range(S // W_DMA):
                c0 = nb * W_DMA
                pts = [
                    psum.tile([BL, NS], f32, name="pt", tag="pt")
                    for _ in range(W_DMA // NS)
                ]
                for g in range(GT):
                    et = epool.tile([128, W_DMA], fmm)
                    nc.sync.dma_start(et[:, :], eop[g * 128:(g + 1) * 128,
                                                    c0:c0 + W_DMA])
                    for ns in range(W_DMA // NS):
                        nc.tensor.matmul(
                            pts[ns][:, :],
                            u_pack[:, g, :],
                            et[:, ns * NS:(ns + 1) * NS],
                            start=(g == 0),
                            stop=(g == GT - 1),
                        )
                for ns in range(W_DMA // NS):
                    nc.any.tensor_copy(
                        energies[:, c0 + ns * NS:c0 + (ns + 1) * NS],
                        pts[ns][:, :],
                    )

            # ---- softmax over s (free dim) ----------------------------------
            m = stats.tile([BL, 1], f32, tag="m")
            nc.vector.reduce_max(m[:, :], energies[:, :],
                                 axis=mybir.AxisListType.X)
            negm = stats.tile([BL, 1], f32, tag="negm")
            nc.vector.tensor_scalar_mul(negm[:, :], m[:, :], -1.0)
            ssum = stats.tile([BL, 1], f32, tag="ssum")
            prob = persist.tile([BL, S], f32, tag="prob")
            nc.scalar.activation(
                prob[:, :],
                energies[:, :],
                mybir.ActivationFunctionType.Exp,
                bias=negm[:, :],
                scale=1.0,
                accum_out=ssum[:, :],
            )
            rs = stats.tile([BL, 1], f32, tag="rs")
            nc.vector.reciprocal(rs[:, :], ssum[:, :])
            nc.vector.tensor_scalar_mul(prob[:, :], prob[:, :], rs[:, :])
            nc.sync.dma_start(attn[:, :], prob[:, :])

    nc.compile()
    return nc


def _pack_inputs(hidden, encoder_outputs, attn_W):
    """Per-core input maps. encoder slice -> [BL, HP, S] zero-padded pack."""
    in_maps = []
    for c in range(NCORES):
        bs = slice(c * BL, (c + 1) * BL)
        buf = np.zeros((BL, HP, S), np.float32)
        np.copyto(buf[:, :H, :], encoder_outputs[:, bs, :].transpose(1, 2, 0))
        in_maps.append({
            "eop": buf.reshape(R, S),
            "ht": np.ascontiguousarray(hidden[0, bs, :].T),
            "wm": attn_W,
        })
    return in_maps


def kernel(hidden, encoder_outputs, question_vector=None, attn_W=None,
           attn_b=None, **_unused):
    import concourse.bass_utils as bass_utils

    hidden = np.asarray(hidden, dtype=np.float32)
    encoder_outputs = np.asarray(encoder_outputs, dtype=np.float32)
    attn_W = np.asarray(attn_W, dtype=np.float32)
    # question_vector and attn_b do not affect the output: question_vector is
    # unused by the reference, and the bias term is constant per softmax row.

    if "nc" not in _cache:
        _cache["nc"] = _build_nc()
    nc = _cache["nc"]

    in_maps = _pack_inputs(hidden, encoder_outputs, attn_W)

    trace = bool(os.environ.get("BASS_KERNEL_TRACE"))
    res = bass_utils.run_bass_kernel_spmd(
        nc, in_maps, core_ids=list(range(NCORES)), trace=trace
    )
    kernel._last_results = res

    out = np.concatenate([res.results[c]["attn"] for c in range(NCORES)], axis=0)
    return out.reshape(1, B, S)


# revision 8
# speedup vs baseline: 1.1831x; 1.1831x over previous
"""Trainium2 Bass kernel for nn_Attn_44281112822483.

Computes attn = softmax_s( hidden[0,b,:] . (W @ encoder_outputs[s,b,:] + bias) )
for S=4096, B=64, H=300, returning [1, B, S] float32.

Key algebraic reduction: with u = hidden[0] @ W  ([B, H]),
    energies[b, s] = sum_h encoder_outputs[s, b, h] * u[b, h] + const(b)
and the per-row constant (hidden . bias) cancels inside the softmax, so the
device only needs one streaming pass over encoder_outputs (315 MB) — a
memory-bound batched dot — plus a tiny [B,H]x[H,H] matmul and a softmax.

Sharding: data-parallel over batch across 8 NeuronCores (8 batches/core);
the 300x300 weight is replicated. Each core's encoder slice is packed
host-side as [8 batches x 320 padded h-rows, 4096 s] so every HBM->SBUF DMA
is a full 128-partition tile with contiguous rows.

The packed u operand [128, 20 row-tiles, 8 batches] is column-selective
(zeros except column b of each row's batch), so ONE K=128 matmul per
(row-tile, column-slice) accumulates every batch's dot products into its own
PSUM row — batch boundaries inside a row-tile need no instruction splits.
"""

import os
import numpy as np

S = 4096
B = 64
H = 300
NCORES = 8
BL = B // NCORES          # batches per core = 8
HP = 320                  # per-batch padded row count (multiple of 64)
R = BL * HP               # 2560 packed rows per core
GT = R // 128             # 20 row-tiles of 128
W_DMA = 2048              # dma tile width (columns of s)
NS = 512                  # matmul moving free dim (one PSUM bank of fp32)
NB = S // W_DMA

# fp32r: single-pass reduced-precision fp32 matmul (~1-2 cycles/row).
# Plain fp32 is near-exact but 4 cycles/row. Toggle if accuracy demands.
USE_F32R = os.environ.get("BASS_KERNEL_F32", "") == ""

_cache = {}


def _build_nc():
    import concourse.bass as bass
    import concourse.tile as tile
    from concourse import bacc, mybir
    from contextlib import ExitStack

    f32 = mybir.dt.float32
    fmm = mybir.dt.float32r if USE_F32R else mybir.dt.float32

    nc = bacc.Bacc(
        "TRN2",
        target_bir_lowering=False,
        debug=False,
        enable_asserts=True,
        num_devices=NCORES,
    )
    eop = nc.dram_tensor("eop", [R, S], fmm, kind="ExternalInput").ap()
    # W pre-chunked host-side over the o-contraction: [3, 128, H] (zero-padded)
    wm3 = nc.dram_tensor("wm3", [3, 128, H], f32, kind="ExternalInput").ap()
    # hidden^T likewise: [3, 128, BL]
    ht3 = nc.dram_tensor("ht3", [3, 128, BL], f32, kind="ExternalInput").ap()
    attn = nc.dram_tensor("attn", [BL, S], f32, kind="ExternalOutput").ap()

    with tile.TileContext(nc) as tc:
        with ExitStack() as ctx:
            consts = ctx.enter_context(tc.tile_pool(name="consts", bufs=1))
            upool = ctx.enter_context(tc.tile_pool(name="upool", bufs=1))
            upsum = ctx.enter_context(
                tc.tile_pool(name="upsum", bufs=1, space="PSUM")
            )
            epool = ctx.enter_context(tc.tile_pool(name="epool", bufs=6))
            psum = ctx.enter_context(tc.tile_pool(name="psum", bufs=6, space="PSUM"))
            persist = ctx.enter_context(tc.tile_pool(name="persist", bufs=1))
            stats = ctx.enter_context(tc.tile_pool(name="stats", bufs=1))

            # ---- consts (scalar HWDGE ring; sync ring is reserved for the
            # encoder stream) ---------------------------------------------
            wm_t = consts.tile([128, 3, H], f32, tag="wmt")
            nc.scalar.dma_start(wm_t[:, :, :], wm3.rearrange("c p h -> p c h"))
            ht_t = consts.tile([128, 3, BL], f32, tag="htt")
            nc.scalar.dma_start(ht_t[:, :, :], ht3.rearrange("c p h -> p c h"))

            # Pre-warm the ACT exp table (one-time table DMA, hidden under
            # the encoder stream).
            warm = stats.tile([1, 1], f32, tag="warm")
            nc.vector.memset(warm[:, :], 0.0)
            nc.scalar.activation(
                warm[:, :], warm[:, :], mybir.ActivationFunctionType.Exp
            )

            # ---- u = hidden @ W -> u_sb [BL, H] (fp32) ----------------------
            u_ps = upsum.tile([BL, H], f32, tag="ups")
            for ci in range(3):
                nc.tensor.matmul(
                    u_ps[:, :],
                    ht_t[:, ci, :],
                    wm_t[:, ci, :],
                    start=(ci == 0),
                    stop=(ci == 2),
                )
            u_sb = upool.tile([BL, H], f32, tag="usb")
            nc.vector.tensor_copy(u_sb[:, :], u_ps[:, :])

            # ---- scatter u into packed layout [128, GT, BL] -----------------
            # u_pack[p, g, j] = u[b, h] when row 128g+p is (b, h) and j == b,
            # else 0.  Column-selective zeros let one PSUM tile accumulate all
            # batches: row j of (lhsT.T @ rhs) sums only batch j's rows.
            u_pack = upool.tile([128, GT, BL], fmm, tag="upack")
            nc.vector.memset(u_pack[:, :, :].bitcast(f32), 0.0)
            for b in range(BL):
                h0 = 0
                while h0 < H:
                    r0 = b * HP + h0
                    g, off = divmod(r0, 128)
                    ln = min(128 - off, H - h0)
                    src = u_sb[b:b + 1, h0:h0 + ln].bitcast(fmm)
                    nc.scalar.dma_start(u_pack[off:off + ln, g, b:b + 1], src)
                    h0 += ln

            # ---- main stream: energies[b, s] --------------------------------
            energies = persist.tile([BL, S], f32, tag="energies")
            # per-block partial maxes, reduced while the stream continues
            pmax = stats.tile([BL, NB], f32, tag="pmax")
            for nb in range(NB):
                c0 = nb * W_DMA
                pts = [
                    psum.tile([BL, NS], f32, name="pt", tag="pt")
                    for _ in range(W_DMA // NS)
                ]
                for g in range(GT):
                    et = epool.tile([128, W_DMA], fmm)
                    nc.sync.dma_start(et[:, :], eop[g * 128:(g + 1) * 128,
                                                    c0:c0 + W_DMA])
                    for ns in range(W_DMA // NS):
                        nc.tensor.matmul(
                            pts[ns][:, :],
                            u_pack[:, g, :],
                            et[:, ns * NS:(ns + 1) * NS],
                            start=(g == 0),
                            stop=(g == GT - 1),
                        )
                for ns in range(W_DMA // NS):
                    nc.any.tensor_copy(
                        energies[:, c0 + ns * NS:c0 + (ns + 1) * NS],
                        pts[ns][:, :],
                    )
                nc.vector.reduce_max(
                    pmax[:, nb:nb + 1],
                    energies[:, c0:c0 + W_DMA],
                    axis=mybir.AxisListType.X,
                )

            # ---- softmax over s (free dim) ----------------------------------
            negm = stats.tile([BL, 1], f32, tag="negm")
            nc.vector.reduce_max(negm[:, :], pmax[:, :],
                                 axis=mybir.AxisListType.X, negate=True)
            ssum = stats.tile([BL, 1], f32, tag="ssum")
            prob = persist.tile([BL, S], f32, tag="prob")
            nc.scalar.activation(
                prob[:, :],
                energies[:, :],
                mybir.ActivationFunctionType.Exp,
                bias=negm[:, :],
                scale=1.0,
                accum_out=ssum[:, :],
            )
            rs = stats.tile([BL, 1], f32, tag="rs")
            nc.vector.reciprocal(rs[:, :], ssum[:, :])
            nc.vector.tensor_scalar_mul(prob[:, :], prob[:, :], rs[:, :])
            nc.scalar.dma_start(attn[:, :], prob[:, :])

    nc.compile()
    return nc


def _pack_inputs(hidden, encoder_outputs, attn_W):
    """Per-core input maps. encoder slice -> [BL, HP, S] zero-padded pack."""
    wm3 = np.zeros((3, 128, H), np.float32)
    for c in range(3):
        kc = min(128, H - c * 128)
        wm3[c, :kc, :] = attn_W[c * 128:c * 128 + kc, :]
    in_maps = []
    for c in range(NCORES):
        bs = slice(c * BL, (c + 1) * BL)
        buf = np.zeros((BL, HP, S), np.float32)
        np.copyto(buf[:, :H, :], encoder_outputs[:, bs, :].transpose(1, 2, 0))
        ht3 = np.zeros((3, 128, BL), np.float32)
        hT = hidden[0, bs, :].T  # [H, BL]
        for ci in range(3):
            kc = min(128, H - ci * 128)
            ht3[ci, :kc, :] = hT[ci * 128:ci * 128 + kc, :]
        in_maps.append({
            "eop": buf.reshape(R, S),
            "ht3": ht3,
            "wm3": wm3,
        })
    return in_maps


def kernel(hidden, encoder_outputs, question_vector=None, attn_W=None,
           attn_b=None, **_unused):
    import concourse.bass_utils as bass_utils

    hidden = np.asarray(hidden, dtype=np.float32)
    encoder_outputs = np.asarray(encoder_outputs, dtype=np.float32)
    attn_W = np.asarray(attn_W, dtype=np.float32)
    # question_vector and attn_b do not affect the output: question_vector is
    # unused by the reference, and the bias term is constant per softmax row.

    if "nc" not in _cache:
        _cache["nc"] = _build_nc()
    nc = _cache["nc"]

    in_maps = _pack_inputs(hidden, encoder_outputs, attn_W)

    trace = bool(os.environ.get("BASS_KERNEL_TRACE"))
    res = bass_utils.run_bass_kernel_spmd(
        nc, in_maps, core_ids=list(range(NCORES)), trace=trace
    )
    kernel._last_results = res

    out = np.concatenate([res.results[c]["attn"] for c in range(NCORES)], axis=0)
    return out.reshape(1, B, S)
